# revision 1
# baseline (speedup 1.0000x reference)
"""ConvAttention Trainium2 kernel.

Data-parallel over batch: 16 examples -> 8 cores x 2 examples.
Per core (all matmuls bf16, fp32 PSUM accumulate):
  key encoder : conv1d(512->1024,k=3)+relu, conv1d(1024->80,k=1), via
                shifted matmuls (SAME padding = edge-clipped partial-range
                PSUM accumulation).
  query encoder: conv1d(80->160,k=3)+relu, conv1d(160->80,k=1)+relu,
                conv1d(80->80,k=1).
  Logits -0.0005*(q2 + k2 - 2qk) are computed (up to a per-row constant q2
  that cancels in softmax and log_softmax) as one K=81 matmul:
  lhsT rows 0..79 = q_enc, row 80 = 1 (padded-weight bias trick);
  rhs rows 0..79 = 0.001*k_enc, row 80 = -500*sum((0.001*k_enc)^2)
  (written via SBUF->SBUF DMA; compute engines can't address partition 80).
  Epilogue ships lnp = log(prior+1e-8) from host:
    z = psum + lnp      (DVE) -> stored as o1
    t = exp(z), accum S1 (ACT) -> stored as o2;  o2 *= 1/S1 per unit (POOL)
    exp(psum), accum S0  (ACT, value discarded; emitted after t-exp so the
                          single batched Ln can't be scheduled early)
    o1 -= ln(S0)         (tail; one Ln for both examples avoids act-table
                          thrash between the exp and ln tables)
  Input DMAs are issued in consumption order (big kconv1 weights split into
  four co-quarters) because the modeled DMA engines drain strictly in issue
  order.
"""

import os

import numpy as np
import ml_dtypes

import concourse.bass as bass
import concourse.tile as tile
from concourse import bacc, mybir
from concourse.bass_utils import run_bass_kernel_spmd

BF = ml_dtypes.bfloat16
F8 = ml_dtypes.float8_e4m3
F32 = mybir.dt.float32
BF16 = mybir.dt.bfloat16
FP8 = mybir.dt.float8e4
W1SCALE = 64.0   # fp8 kconv1 weight prescale; descale folded into wk2/kb1

N_CORES = 8
BPC = 2          # examples per core
TQ = 800
TK = 200
N_MEL = 80
N_TEXT = 512
N_ATTN = 80
C1K = 1024       # key conv1 out channels
C1Q = 160        # query conv1 out channels
NU = 7           # qk row chunks per example (6x128 + 32)

Act = mybir.ActivationFunctionType
Alu = mybir.AluOpType

LAST_RESULT = None
_DBG = int(os.environ.get("KDBG", "4"))
_FP8 = os.environ.get("KFP8", "1") == "1"  # fp8+DoubleRow for kconv1
_REPS = int(os.environ.get("KREPS", "1"))  # body replication for benchmarking


def _build_program():
    nc = bacc.Bacc("TRN2", target_bir_lowering=False, debug=False,
                   num_devices=N_CORES)

    # ---- DRAM I/O ----
    kdt = FP8 if _FP8 else BF16
    keys_d = nc.dram_tensor("keys", [BPC, N_TEXT, TK], kdt, kind="ExternalInput").ap()
    q_d = nc.dram_tensor("queries", [BPC, N_MEL, TQ], BF16, kind="ExternalInput").ap()
    lnp_d = nc.dram_tensor("lnp", [BPC, TQ, TK], F32, kind="ExternalInput").ap()
    if _FP8:
        wk1_d = nc.dram_tensor("wk1", [2, 128, 2, 3, C1K], FP8,
                               kind="ExternalInput").ap()
    else:
        wk1_d = nc.dram_tensor("wk1", [N_TEXT, 3, C1K], BF16,
                               kind="ExternalInput").ap()
    wk2_d = nc.dram_tensor("wk2", [C1K, N_ATTN], BF16, kind="ExternalInput").ap()
    wq1_d = nc.dram_tensor("wq1", [N_MEL, 3, C1Q], BF16, kind="ExternalInput").ap()
    wq2_d = nc.dram_tensor("wq2", [N_MEL, 2, N_MEL], BF16, kind="ExternalInput").ap()
    wq3_d = nc.dram_tensor("wq3", [N_MEL, N_ATTN], BF16, kind="ExternalInput").ap()
    bias_d = nc.dram_tensor("biases", [128, 13], F32, kind="ExternalInput").ap()
    attn_d = nc.dram_tensor("attn", [BPC, TQ, TK], F32, kind="ExternalOutput").ap()
    logp_d = nc.dram_tensor("logp", [BPC, TQ, TK], F32, kind="ExternalOutput").ap()

    with tile.TileContext(nc) as tc:
        with (
            tc.tile_pool(name="singles", bufs=1) as singles,
            tc.tile_pool(name="acts", bufs=2) as acts,
            tc.tile_pool(name="epi", bufs=2) as epi,
            tc.tile_pool(name="small", bufs=4) as small,
            tc.tile_pool(name="psC", bufs=4, space="PSUM") as psC,
            tc.tile_pool(name="psQK", bufs=4, space="PSUM") as psQK,
        ):
            # ---- input DMAs, in consumption order ----
            q_sb = [acts.tile([N_MEL, TQ], BF16, name=f"q_sb{e}", tag="q_sb")
                    for e in range(2)]
            nc.sync.dma_start(out=q_sb[0], in_=q_d[0])
            wq1_sb = singles.tile([N_MEL, 3, C1Q], BF16)
            nc.sync.dma_start(out=wq1_sb, in_=wq1_d)
            wq2_sb = singles.tile([N_MEL, 2, N_MEL], BF16)
            nc.sync.dma_start(out=wq2_sb, in_=wq2_d)
            wq3_sb = singles.tile([N_MEL, N_ATTN], BF16)
            nc.sync.dma_start(out=wq3_sb, in_=wq3_d)
            bias_sb = singles.tile([128, 13], F32)
            nc.sync.dma_start(out=bias_sb, in_=bias_d)
            keys_sb = [acts.tile([128, 4, TK], kdt, name=f"keys_sb{e}",
                                 tag="keys_sb") for e in range(2)]
            nc.sync.dma_start(out=keys_sb[0],
                              in_=keys_d[0].rearrange("(c p) t -> p c t", p=128))
            nc.sync.dma_start(out=q_sb[1], in_=q_d[1])
            # kconv1 weights, chunked DMAs (DMA APs are limited to 3 dims)
            if _FP8:
                wk1_sb = singles.tile([128, 2, 2, 3, C1K], FP8)
                for j in range(2):
                    nc.sync.dma_start(out=wk1_sb[:, j], in_=wk1_d[j])
            else:
                wk1_sb = singles.tile([128, 4, 3, C1K], BF16)
                wk1_r = wk1_d.rearrange("(c p) k m -> p c k m", p=128)
                for ci in range(4):
                    nc.sync.dma_start(out=wk1_sb[:, ci], in_=wk1_r[:, ci])
            wk2_sb = singles.tile([128, 8, N_ATTN], BF16)
            nc.sync.dma_start(out=wk2_sb, in_=wk2_d.rearrange("(c p) m -> p c m", p=128))
            nc.sync.dma_start(out=keys_sb[1],
                              in_=keys_d[1].rearrange("(c p) t -> p c t", p=128))
            lnp_all = [epi.tile([128, NU, TK], F32, name=f"lnp_all{e}",
                                tag="lnp_all") for e in range(2)]

            def load_lnp(e):
                nc.sync.dma_start(out=lnp_all[e][:, 0:6],
                                  in_=lnp_d[e, 0:768].rearrange("(c p) t -> p c t", p=128))
                nc.sync.dma_start(out=lnp_all[e][0:32, 6], in_=lnp_d[e, 768:TQ])

            ones80 = singles.tile([N_ATTN, 1], BF16)
            nc.vector.memset(ones80, 1.0)
            k_aug = singles.tile([N_ATTN, BPC, TK], BF16)
            onesrow = singles.tile([1, 128], BF16)
            nc.vector.memset(onesrow, 1.0)
            # one S0 tile for both examples -> one Ln at the very end
            S0s = singles.tile([128, BPC, NU], F32)
            nc.vector.memset(S0s, 1.0)

            def qconv(e):
                y1q = acts.tile([N_MEL, 2, TQ], BF16, name="y1q", tag="y1q")
                q_aug = acts.tile([N_ATTN, TQ], BF16, name="q_aug", tag="q_aug")
                # conv1 k=3: 80 -> 160 as two co-halves of 80
                for t0 in (0, 400):
                    for h in (0, 1):
                        co_sl = slice(h * 80, (h + 1) * 80)
                        ps = psC.tile([N_MEL, 400], F32, name="psq1", tag="conv")
                        nc.tensor.matmul(ps, wq1_sb[:, 1, co_sl],
                                         q_sb[e][:, t0:t0 + 400],
                                         start=True, stop=False)
                        if t0 == 0:
                            nc.tensor.matmul(ps[:, 1:400], wq1_sb[:, 0, co_sl],
                                             q_sb[e][:, 0:399],
                                             start=False, stop=False)
                        else:
                            nc.tensor.matmul(ps, wq1_sb[:, 0, co_sl],
                                             q_sb[e][:, t0 - 1:t0 + 399],
                                             start=False, stop=False)
                        if t0 + 400 == TQ:
                            nc.tensor.matmul(ps[:, 0:399], wq1_sb[:, 2, co_sl],
                                             q_sb[e][:, t0 + 1:TQ],
                                             start=False, stop=True)
                        else:
                            nc.tensor.matmul(ps, wq1_sb[:, 2, co_sl],
                                             q_sb[e][:, t0 + 1:t0 + 401],
                                             start=False, stop=True)
                        nc.vector.tensor_scalar(out=y1q[:, h, t0:t0 + 400], in0=ps,
                                                scalar1=bias_sb[0:N_MEL, 9 + h:10 + h],
                                                scalar2=0.0, op0=Alu.add, op1=Alu.max)
                # conv2 k=1: 160 -> 80, relu
                y2q = acts.tile([N_MEL, TQ], BF16, name="y2q", tag="y2q")
                for t0 in (0, 400):
                    ps = psC.tile([N_MEL, 400], F32, name="psq2", tag="conv")
                    nc.tensor.matmul(ps, wq2_sb[:, 0], y1q[:, 0, t0:t0 + 400],
                                     start=True, stop=False)
                    nc.tensor.matmul(ps, wq2_sb[:, 1], y1q[:, 1, t0:t0 + 400],
                                     start=False, stop=True)
                    nc.vector.tensor_scalar(out=y2q[:, t0:t0 + 400], in0=ps,
                                            scalar1=bias_sb[0:N_MEL, 11:12],
                                            scalar2=0.0, op0=Alu.add, op1=Alu.max)
                # conv3 k=1: 80 -> 80
                for t0 in (0, 400):
                    ps = psC.tile([N_ATTN, 400], F32, name="psq3", tag="conv")
                    nc.tensor.matmul(ps, wq3_sb, y2q[:, t0:t0 + 400],
                                     start=True, stop=True)
                    nc.vector.tensor_scalar_add(q_aug[:, t0:t0 + 400], ps,
                                                bias_sb[0:N_ATTN, 12:13])
                return q_aug

            def kconv(e):
                # conv1 k=3: 512 -> 1024
                y1k = []
                DR = mybir.MatmulPerfMode.DoubleRow
                for co in range(8):
                    ps = psC.tile([128, TK], F32, name="psk1", tag="conv")
                    co_sl = slice(co * 128, (co + 1) * 128)
                    if _FP8:
                        # DoubleRow: K=256 per matmul, both operands [128,2,*]
                        for j in range(2):
                            ksl = keys_sb[e][:, 2 * j:2 * j + 2]
                            nc.tensor.matmul(ps, wk1_sb[:, j, :, 1, co_sl], ksl,
                                             start=(j == 0), stop=False,
                                             perf_mode=DR)
                            nc.tensor.matmul(ps[:, 1:TK], wk1_sb[:, j, :, 0, co_sl],
                                             ksl[:, :, 0:TK - 1],
                                             start=False, stop=False, perf_mode=DR)
                            nc.tensor.matmul(ps[:, 0:TK - 1], wk1_sb[:, j, :, 2, co_sl],
                                             ksl[:, :, 1:TK],
                                             start=False, stop=(j == 1),
                                             perf_mode=DR)
                    else:
                        for ci in range(4):
                            nc.tensor.matmul(ps, wk1_sb[:, ci, 1, co_sl],
                                             keys_sb[e][:, ci],
                                             start=(ci == 0), stop=False)
                            nc.tensor.matmul(ps[:, 1:TK], wk1_sb[:, ci, 0, co_sl],
                                             keys_sb[e][:, ci, 0:TK - 1],
                                             start=False, stop=False)
                            nc.tensor.matmul(ps[:, 0:TK - 1], wk1_sb[:, ci, 2, co_sl],
                                             keys_sb[e][:, ci, 1:TK],
                                             start=False, stop=(ci == 3))
                    yt = acts.tile([128, TK], BF16, name=f"y1k{co}", tag=f"y1k{co}")
                    nc.vector.tensor_scalar(out=yt, in0=ps,
                                            scalar1=bias_sb[:, co:co + 1],
                                            scalar2=0.0, op0=Alu.add, op1=Alu.max)
                    y1k.append(yt)
                # conv2 k=1: 1024 -> 80, scaled by 1e-3 into k_aug
                ps2 = psC.tile([N_ATTN, TK], F32, name="psk2", tag="conv")
                for ci in range(8):
                    nc.tensor.matmul(ps2, wk2_sb[:, ci], y1k[ci],
                                     start=(ci == 0), stop=(ci == 7))
                nc.vector.tensor_scalar(out=k_aug[:, e], in0=ps2,
                                        scalar1=0.001, scalar2=bias_sb[0:N_ATTN, 8:9],
                                        op0=Alu.mult, op1=Alu.add)
                # k2 row: -500 * sum_c (0.001*k_enc)^2, DMA'd into partition 80
                ksq = acts.tile([N_ATTN, TK], BF16, name="ksq", tag="ksq")
                nc.gpsimd.tensor_mul(ksq, k_aug[:, e], k_aug[:, e])
                psk2r = psC.tile([1, TK], F32, name="psk2r", tag="conv")
                nc.tensor.matmul(psk2r, ones80, ksq, start=True, stop=True)
                k2row = acts.tile([1, TK], BF16, name="k2row", tag="k2row")
                nc.vector.tensor_scalar_mul(k2row, psk2r, -500.0)
                return k2row

            def attention(e, q_aug, k2row, state):
                if e == 0:
                    # issued here so they sit behind k2row(0) in the serial
                    # DMA stream but ahead of the e0 output stores
                    load_lnp(0)
                    load_lnp(1)
                o1_all = epi.tile([128, NU, TK], F32, name="o1_all", tag="o1_all")
                o2_all = epi.tile([128, NU, TK], F32, name="o2_all", tag="o2_all")
                for u in range(NU):
                    a = u * 128
                    m = min(128, TQ - a)
                    ps = psQK.tile([128, TK], F32, name="psqk", tag="qk")
                    nc.tensor.matmul(ps[:m], q_aug[:, a:a + m], k_aug[:, e],
                                     start=True, stop=False)
                    nc.tensor.matmul(ps[:m], onesrow[:, :m], k2row,
                                     start=False, stop=True)
                    nc.vector.tensor_add(o1_all[:m, u], ps[:m], lnp_all[e][:m, u])
                    S1 = small.tile([128, 1], F32, name="S1", tag="S1")
                    nc.scalar.activation(out=o2_all[:m, u], in_=o1_all[:m, u],
                                         func=Act.Exp, accum_out=S1[:m])
                    # S0 exp second, so the tail Ln (reads S0s of both
                    # examples) can't be scheduled before the last t-exp
                    sdump = small.tile([128, TK], F32, name="sdump", tag="sdump")
                    nc.scalar.activation(out=sdump[:m], in_=ps[:m], func=Act.Exp,
                                         accum_out=S0s[:m, e, u:u + 1])
                    r1 = small.tile([128, 1], F32, name="r1", tag="r1")
                    nc.vector.reciprocal(r1[:m], S1[:m])
                    nc.gpsimd.tensor_scalar_mul(o2_all[:m, u], o2_all[:m, u],
                                                r1[:m])
                    if u in (1, 3, 5):
                        c0 = u - 1
                        nc.sync.dma_start(
                            out=attn_d[e, c0 * 128:(u + 1) * 128].rearrange(
                                "(c p) t -> p c t", p=128),
                            in_=o2_all[:, c0:u + 1])
                    elif u == 6:
                        nc.sync.dma_start(out=attn_d[e, 768:TQ],
                                          in_=o2_all[0:32, 6])
                state[e] = o1_all

            def logp_tail(state):
                lnS0s = singles.tile([128, BPC, NU], F32)
                nc.scalar.activation(out=lnS0s, in_=S0s, func=Act.Ln)
                for e in range(2):
                    o1_all = state[e]
                    for u in range(NU):
                        m = min(128, TQ - u * 128)
                        eng = nc.gpsimd if u % 2 else nc.vector
                        eng.tensor_scalar_sub(o1_all[:m, u], o1_all[:m, u],
                                              lnS0s[:m, e, u:u + 1])
                        if u in (1, 3, 5):
                            c0 = u - 1
                            nc.sync.dma_start(
                                out=logp_d[e, c0 * 128:(u + 1) * 128].rearrange(
                                    "(c p) t -> p c t", p=128),
                                in_=o1_all[:, c0:u + 1])
                        elif u == 6:
                            nc.sync.dma_start(out=logp_d[e, 768:TQ],
                                              in_=o1_all[0:32, 6])

            for _rep in range(_REPS):
                state = {}
                q_aug0 = qconv(0)
                q_aug1 = qconv(1)
                k2r0 = kconv(0)
                if _DBG >= 2:
                    attention(0, q_aug0, k2r0, state)
                k2r1 = kconv(1)
                if _DBG >= 2:
                    attention(1, q_aug1, k2r1, state)
                if _DBG >= 3:
                    logp_tail(state)

    nc.compile()
    return nc


_NC = None


def _get_nc():
    global _NC
    if _NC is None:
        _NC = _build_program()
    return _NC


def prepare_in_maps(queries, keys, attn_prior,
                    kW1, kb1, kW2, kb2, qW1, qb1, qW2, qb2, qW3, qb3):
    kb1 = np.float32(kb1)
    if _FP8:
        # [co, ci, k] -> [j, p, i, k, co] with ci = 256j + 128i + p, x64
        wk1 = np.ascontiguousarray(
            np.transpose((np.float32(kW1) * W1SCALE).reshape(C1K, 2, 2, 128, 3),
                         (1, 3, 2, 4, 0))).astype(F8)
        wk2 = np.ascontiguousarray(kW2[:, :, 0].T / W1SCALE).astype(BF)
        kb1 = kb1 * W1SCALE
    else:
        wk1 = np.ascontiguousarray(np.transpose(kW1, (1, 2, 0))).astype(BF)
        wk2 = np.ascontiguousarray(kW2[:, :, 0].T).astype(BF)
    wq1 = np.ascontiguousarray(np.transpose(qW1, (1, 2, 0))).astype(BF)
    wq2 = np.ascontiguousarray(
        np.transpose(qW2[:, :, 0].T.reshape(2, N_MEL, N_MEL), (1, 0, 2))).astype(BF)
    wq3 = np.ascontiguousarray(qW3[:, :, 0].T).astype(BF)
    biases = np.zeros((128, 13), np.float32)
    biases[:, 0:8] = kb1.reshape(8, 128).T
    biases[0:N_ATTN, 8] = 0.001 * np.float32(kb2)
    biases[0:N_MEL, 9] = np.float32(qb1)[0:80]
    biases[0:N_MEL, 10] = np.float32(qb1)[80:160]
    biases[0:N_MEL, 11] = np.float32(qb2)
    biases[0:N_ATTN, 12] = np.float32(qb3)
    shared = dict(wk1=wk1, wk2=wk2, wq1=wq1, wq2=wq2, wq3=wq3, biases=biases)

    keys_b = np.asarray(keys).astype(F8 if _FP8 else BF)
    q_b = np.asarray(queries).astype(BF)
    lnp = np.log(np.asarray(attn_prior) + np.float32(1e-8)).astype(np.float32)

    in_maps = []
    for c in range(N_CORES):
        sl = slice(c * BPC, (c + 1) * BPC)
        in_maps.append(dict(
            keys=np.ascontiguousarray(keys_b[sl]),
            queries=np.ascontiguousarray(q_b[sl]),
            lnp=np.ascontiguousarray(lnp[sl]),
            **shared,
        ))
    return in_maps


def kernel(queries, keys, query_lens, mask, attn_prior,
           kW1, kb1, kW2, kb2, qW1, qb1, qW2, qb2, qW3, qb3,
           trace=False):
    global LAST_RESULT
    nc = _get_nc()
    in_maps = prepare_in_maps(queries, keys, attn_prior, kW1, kb1, kW2, kb2,
                              qW1, qb1, qW2, qb2, qW3, qb3)

    res = run_bass_kernel_spmd(nc, in_maps, core_ids=list(range(N_CORES)),
                               trace=trace)
    LAST_RESULT = res

    B = N_CORES * BPC
    attn = np.empty((B, 1, TQ, TK), np.float32)
    logp = np.empty((B, 1, TQ, TK), np.float32)
    for c in range(N_CORES):
        attn[c * BPC:(c + 1) * BPC, 0] = res.results[c]["attn"]
        logp[c * BPC:(c + 1) * BPC, 0] = res.results[c]["logp"]
    return attn, logp

